# revision 11
# baseline (speedup 1.0000x reference)
"""Trainium2 Bass kernel for nn_CrossMatrix.

Computes, per batch row b (B = 65536 rows total):
    x1   = [1, x[b]]                      (65,)
    y[j] = sum_{a,c} W[j, a*65+c] * x1[a] * x1[c] + bias[j]     (64,)
    out  = LayerNorm(y) * gamma + beta  -> reshape (8, 8)

Sharding: pure data parallel, batch split across 8 cores (8192 rows each).

Default algorithm (_build_bass_v3, "eigen-square"): the host symmetrizes each
quadratic form Q_j and eigendecomposes it, so y_quad_j = sum_k s_jk*(u'_jk.x)^2
with u' = sqrt(|lambda|)-scaled eigenvectors (exact, rank 64).  On device the
batched-elementwise multiply that a direct Z = x.V / y = sum_a x_a*Z epilogue
would need (VectorE-bound at 1x rate) is replaced by a *square*, which needs no
second operand and therefore works in feature-major layout:

  per 512-row block, per 128-row (j,k)-chunk (32 chunks):
    G^T    [128,512] = U2_chunk.T @ XT      TensorE (fp16, 1 cyc/row)
    (G^T)^2           square on PSUM->SBUF eviction   ScalarE
    y^T    [64,512] += R_chunk.T @ (G^T)^2  TensorE, signed selection matrix,
                                            PSUM-accumulated over chunks
    y^T            += L65.T @ XT65          TensorE (linear + const via ones row)
    transpose y^T back to batch-major       TensorE
    LayerNorm (bn_stats/bn_aggr/rsqrt/tensor_scalar)  VectorE (+ScalarE sqrt)

so both contractions AND the k-reduction run on TensorE, and the only
per-element pass over the 4096-wide intermediate is the ScalarE square.
x is transposed once per 128-row tile via TensorE (identity transpose).

The older _build_bass (v1) batch-major formulation (stage-2 multiply+reduce on
VectorE) is kept for reference/fallback; it is ~2.3x slower (VectorE-bound).

Notes that matter on this stack:
 - built with bacc.Bacc + finalize(): its split_sync_waits/
   move_matmul_waits_to_ldweights passes legalize multi-semaphore waits
   (walrus rejects >1 wait per compute instruction).
 - fp32 matmuls are 4 cyc/row on TRN2; fp16/bf16 are 1 cyc/row.  fp16 is used
   end-to-end (10 mantissa bits) -> ~7e-4 max rel err vs the fp32 reference.
 - float32r executed but returned wrong data on HW here; avoided.
 - VectorE cannot read PSUM twice in one op, GPSIMD cannot read PSUM at all,
   and ScalarE squares straight out of PSUM - hence the ScalarE choice.
"""

import numpy as np

import concourse.bass as bass
from concourse import bacc
import concourse.mybir as mybir
import concourse.tile as tile
from concourse.bass_utils import run_bass_kernel_spmd
from concourse.masks import make_identity

# Problem constants (hardcoded per harness contract).
B = 65536
N = 64          # features out (8*8)
NX = 64         # features in (x per row)
INNER = 4225    # (N+1)**2
EPS = 1e-6
N_CORES = 8
ROWS_PER_CORE = B // N_CORES  # 8192
P = 128                       # partitions / batch tile rows

F32 = mybir.dt.float32
F32R = mybir.dt.float32r
BF16 = mybir.dt.bfloat16
FP16 = mybir.dt.float16
V3DT = FP16  # dtype for the whole v3 PE datapath
G2DT = FP16  # dtype for squared projections + selection matrix

_CACHE = {}


def _build_bass(rows_per_core, apply_gamma, apply_beta):
    ntiles = rows_per_core // P
    nc = bacc.Bacc(None, target_bir_lowering=False)

    xs = nc.declare_dram_parameter("xs", [rows_per_core, NX], F32, isOutput=False)
    vq = nc.declare_dram_parameter("vq", [NX, N * NX], BF16, isOutput=False)
    lw = nc.declare_dram_parameter("lw", [NX, N], BF16, isOutput=False)
    cb = nc.declare_dram_parameter("cb", [P, N], F32, isOutput=False)
    if apply_gamma:
        gm = nc.declare_dram_parameter("gm", [P, N], F32, isOutput=False)
    if apply_beta:
        bt = nc.declare_dram_parameter("bt", [P, N], F32, isOutput=False)
    out = nc.declare_dram_parameter("out", [rows_per_core, N], F32, isOutput=True)

    with (
        tile.TileContext(nc) as tc,
        tc.tile_pool(name="consts", bufs=1) as consts,
        tc.tile_pool(name="xpool", bufs=3) as xpool,
        tc.tile_pool(name="xtpool", bufs=2) as xtpool,
        tc.tile_pool(name="ppool", bufs=3) as ppool,
        tc.tile_pool(name="ypool", bufs=3) as ypool,
        tc.tile_pool(name="opool", bufs=3) as opool,
        tc.tile_pool(name="stat", bufs=4) as stat,
        tc.tile_pool(name="zpsum", bufs=3, space="PSUM") as zpsum,
        tc.tile_pool(name="ypsum", bufs=2, space="PSUM") as ypsum,
        tc.tile_pool(name="tpsum", bufs=1, space="PSUM") as tpsum,
        tc.tile_pool(name="scrps", bufs=1, space="PSUM") as scrps,
    ):
        # --- constants in SBUF ---
        ident = consts.tile([P, P], F32)
        make_identity(nc, ident)
        v_sb = consts.tile([NX, N * NX], BF16)
        nc.gpsimd.dma_start(out=v_sb, in_=vq[:, :])
        l_sb = consts.tile([NX, N], BF16)
        nc.gpsimd.dma_start(out=l_sb, in_=lw[:, :])
        c_sb = consts.tile([P, N], F32)
        nc.gpsimd.dma_start(out=c_sb, in_=cb[:, :])
        if apply_gamma:
            g_sb = consts.tile([P, N], F32)
            nc.gpsimd.dma_start(out=g_sb, in_=gm[:, :])
        if apply_beta:
            b_sb = consts.tile([P, N], F32)
            nc.gpsimd.dma_start(out=b_sb, in_=bt[:, :])
        eps_sb = consts.tile([P, 1], F32)
        nc.vector.memset(eps_sb, EPS)

        # --- warm-up absorbers ---
        # The PE weight-load slot only fits one semaphore wait, so make each
        # engine observe every const producer once, via single-dep ops, before
        # the steady-state loop (keeps every later instruction at <=1 fresh
        # cross-engine wait).
        scr_t = scrps.tile([P, P], F32, tag="scr")
        nc.tensor.transpose(scr_t, ident, ident)  # PE observes gpsimd memset
        scr_m = scrps.tile([1, 1], F32, tag="scr")
        nc.tensor.matmul(scr_m, lhsT=v_sb[:, 0:1], rhs=v_sb[:, 0:1],
                         start=True, stop=True)  # PE observes V dma
        scr_m2 = scrps.tile([1, 1], F32, tag="scr")
        nc.tensor.matmul(scr_m2, lhsT=l_sb[:, 0:1], rhs=l_sb[:, 0:1],
                         start=True, stop=True)  # PE observes L dma
        scr_v = consts.tile([P, 1], F32)
        nc.vector.tensor_copy(scr_v, c_sb[:, 0:1])  # DVE observes C dma
        if apply_gamma:
            nc.vector.tensor_copy(scr_v, g_sb[:, 0:1])
        if apply_beta:
            nc.vector.tensor_copy(scr_v, b_sb[:, 0:1])

        for t in range(ntiles):
            r0 = t * P
            # load x tile [128, 64]
            x_sb = xpool.tile([P, NX], F32)
            nc.gpsimd.dma_start(out=x_sb, in_=xs[r0 : r0 + P, :])

            # transpose -> xt [64, 128] (TensorE, lands in PSUM, evict to SBUF)
            xt_ps = tpsum.tile([NX, P], F32)
            nc.tensor.transpose(xt_ps, x_sb, ident)
            xt_sb = xtpool.tile([NX, P], BF16)
            nc.vector.tensor_copy(out=xt_sb, in_=xt_ps)

            # DVE-local copy of x so the stage-2 multiplies never wait on DMA
            xf_sb = xtpool.tile([P, NX], F32)
            nc.vector.tensor_copy(out=xf_sb, in_=x_sb)

            yq_sb = ypool.tile([P, N], F32)

            # stage-1 + stage-2 per 512-wide chunk (8 j-values x 64 a)
            for k in range(8):
                z_ps = zpsum.tile([P, 512], F32)
                nc.tensor.matmul(
                    z_ps,
                    lhsT=xt_sb[:, :],
                    rhs=v_sb[:, k * 512 : (k + 1) * 512],
                    start=True,
                    stop=True,
                )
                # P = Z * x (broadcast x over the 8 j's in this chunk)
                p_sb = ppool.tile([P, 8, NX], F32)
                x_bc = xf_sb[:, :].unsqueeze(1).broadcast_to((P, 8, NX))
                nc.vector.tensor_mul(
                    p_sb,
                    z_ps[:, :].rearrange("p (j a) -> p j a", a=NX),
                    x_bc,
                )
                # y_quad[:, 8k:8k+8] = sum_a P
                nc.vector.reduce_sum(
                    out=yq_sb[:, k * 8 : (k + 1) * 8],
                    in_=p_sb,
                    axis=mybir.AxisListType.X,
                )

            # linear part on TensorE: ylin = xT.T @ L  -> [128, 64]
            yl_ps = ypsum.tile([P, N], F32)
            nc.tensor.matmul(
                yl_ps,
                lhsT=xt_sb[:, :],
                rhs=l_sb[:, :],
                start=True,
                stop=True,
            )

            # y = yq + ylin + const
            y_sb = ypool.tile([P, N], F32)
            nc.vector.tensor_add(y_sb, yq_sb, yl_ps)
            nc.vector.tensor_add(y_sb, y_sb, c_sb)

            # LayerNorm over the 64 features
            st_sb = stat.tile([P, 6], F32)
            nc.vector.bn_stats(out=st_sb, in_=y_sb)
            mv_sb = stat.tile([P, 2], F32)
            nc.vector.bn_aggr(out=mv_sb, in_=st_sb)
            # rstd = 1/sqrt(var + eps)
            sd_sb = stat.tile([P, 1], F32)
            nc.scalar.activation(
                out=sd_sb,
                in_=mv_sb[:, 1:2],
                func=mybir.ActivationFunctionType.Sqrt,
                bias=eps_sb,
                scale=1.0,
            )
            rs_sb = stat.tile([P, 1], F32)
            nc.vector.reciprocal(out=rs_sb, in_=sd_sb)

            o_sb = opool.tile([P, N], F32)
            nc.vector.tensor_scalar(
                out=o_sb,
                in0=y_sb,
                scalar1=mv_sb[:, 0:1],
                scalar2=rs_sb,
                op0=mybir.AluOpType.subtract,
                op1=mybir.AluOpType.mult,
            )
            if apply_gamma:
                nc.vector.tensor_mul(o_sb, o_sb, g_sb)
            if apply_beta:
                nc.vector.tensor_add(o_sb, o_sb, b_sb)

            nc.gpsimd.dma_start(out=out[r0 : r0 + P, :], in_=o_sb)

    if not nc.is_finalized():
        nc.finalize()
    return nc


def _build_bass_v3(rows_per_core, apply_gamma, apply_beta):
    """Eigendecomposition formulation, feature-major.

    Host factors each symmetrized quadratic form: y_quad_j = sum_k s_jk*(u'_jk.x)^2
    with u' = sqrt(|lambda|)-scaled eigenvectors. On device, per 512-row block:
      G^T chunks [128 (j,k) rows, 512 batch] = U2_chunk.T @ XT   (TensorE)
      square G^T during PSUM->SBUF eviction                      (ScalarE/VectorE)
      y^T += R_chunk.T @ (G^T)^2  (signed selection, PSUM accum) (TensorE)
      y^T += L65.T @ XT65         (linear + const via ones row)  (TensorE)
      transpose y^T back to batch-major (TensorE) + LayerNorm    (VectorE/ScalarE)
    This keeps the per-element work (the squares) on ScalarE/VectorE at one pass
    over G, and does both contractions AND the k-reduction on TensorE.
    """
    ntiles = rows_per_core // P
    nblocks = rows_per_core // 512
    NK = N * NX  # 4096 (j,k) pairs
    nc = bacc.Bacc(None, target_bir_lowering=False)

    xs = nc.declare_dram_parameter("xs", [rows_per_core, NX], F32, isOutput=False)
    u2 = nc.declare_dram_parameter("u2", [NX, NK], V3DT, isOutput=False)
    rsel = nc.declare_dram_parameter("rsel", [P, 32 * N], G2DT, isOutput=False)
    l65 = nc.declare_dram_parameter("l65", [NX + 1, N], V3DT, isOutput=False)
    if apply_gamma:
        gm = nc.declare_dram_parameter("gm", [P, N], F32, isOutput=False)
    if apply_beta:
        bt = nc.declare_dram_parameter("bt", [P, N], F32, isOutput=False)
    out = nc.declare_dram_parameter("out", [rows_per_core, N], F32, isOutput=True)

    with (
        tile.TileContext(nc) as tc,
        tc.tile_pool(name="consts", bufs=1) as consts,
        tc.tile_pool(name="xpool", bufs=4) as xpool,
        tc.tile_pool(name="g2pool", bufs=4) as g2pool,
        tc.tile_pool(name="ysb", bufs=2) as ysb,
        tc.tile_pool(name="opool", bufs=4) as opool,
        tc.tile_pool(name="stat", bufs=8) as stat,
        tc.tile_pool(name="gps", bufs=2, space="PSUM") as gps,
        tc.tile_pool(name="ytps", bufs=2, space="PSUM") as ytps,
        tc.tile_pool(name="tps", bufs=1, space="PSUM") as tps,
        tc.tile_pool(name="ybps", bufs=1, space="PSUM") as ybps,
    ):
        ident = consts.tile([P, P], F32)
        make_identity(nc, ident)
        u2_sb = consts.tile([NX, NK], V3DT)
        nc.sync.dma_start(out=u2_sb, in_=u2[:, :])
        rs_sb = consts.tile([P, 32 * N], G2DT)
        nc.sync.dma_start(out=rs_sb, in_=rsel[:, :])
        l65_sb = consts.tile([NX + 1, N], V3DT)
        nc.sync.dma_start(out=l65_sb, in_=l65[:, :])
        if apply_gamma:
            g_sb = consts.tile([P, N], F32)
            nc.sync.dma_start(out=g_sb, in_=gm[:, :])
        if apply_beta:
            b_sb = consts.tile([P, N], F32)
            nc.sync.dma_start(out=b_sb, in_=bt[:, :])
        eps_sb = consts.tile([P, 1], F32)
        nc.vector.memset(eps_sb, EPS)

        # transposed x for the whole core slice, with a trailing ones row
        xt65 = consts.tile([NX + 1, rows_per_core], V3DT)
        nc.vector.memset(xt65[NX : NX + 1, :], 1.0)

        # warm-up absorbers (keep fresh cross-engine waits at <=1 per inst)
        scr_t = gps.tile([P, P], F32, tag="g")
        nc.tensor.transpose(scr_t, ident, ident)
        scr_m = gps.tile([1, 1], F32, tag="g")
        nc.tensor.matmul(scr_m, lhsT=u2_sb[:, 0:1], rhs=u2_sb[:, 0:1],
                         start=True, stop=True)
        scr_m2 = gps.tile([1, 1], F32, tag="g")
        nc.tensor.matmul(scr_m2, lhsT=rs_sb[:, 0:1], rhs=rs_sb[:, 0:1],
                         start=True, stop=True)
        scr_m3 = gps.tile([1, 1], F32, tag="g")
        nc.tensor.matmul(scr_m3, lhsT=l65_sb[:, 0:1], rhs=l65_sb[:, 0:1],
                         start=True, stop=True)

        for t in range(ntiles):
            r0 = t * P
            x_sb = xpool.tile([P, NX], F32)
            nc.sync.dma_start(out=x_sb, in_=xs[r0 : r0 + P, :])
            xt_ps = tps.tile([NX, P], F32)
            nc.tensor.transpose(xt_ps, x_sb, ident)
            nc.vector.tensor_copy(out=xt65[0:NX, r0 : r0 + P], in_=xt_ps)

        # per-block split of the 16 chunk-pair squarings across engines:
        # first ACT_PAIRS pairs -> ScalarE squares straight out of PSUM;
        # the rest -> VectorE copies PSUM->SBUF (fp16), then GPSIMD (or
        # VectorE for DVE_PAIRS of them) squares in SBUF.
        ACT_PAIRS = 16
        DVE_PAIRS = 0

        for s in range(nblocks):
            c0 = s * 512
            yt_ps = ytps.tile([N, 512], F32)
            for pr in range(16):
                g_ps = gps.tile([P, 1024], F32, tag="g")
                for h in range(2):
                    c = 2 * pr + h
                    nc.tensor.matmul(
                        g_ps[:, h * 512 : (h + 1) * 512],
                        lhsT=u2_sb[:, c * P : (c + 1) * P],
                        rhs=xt65[0:NX, c0 : c0 + 512],
                        start=True,
                        stop=True,
                    )
                g2_sb = g2pool.tile([P, 1024], G2DT)
                if pr < ACT_PAIRS:
                    nc.scalar.square(g2_sb, g_ps)
                else:
                    gc_sb = g2pool.tile([P, 1024], G2DT, tag="gc")
                    nc.vector.tensor_copy(out=gc_sb, in_=g_ps)
                    if pr < ACT_PAIRS + DVE_PAIRS:
                        nc.vector.tensor_mul(g2_sb, gc_sb, gc_sb)
                    else:
                        nc.gpsimd.tensor_mul(g2_sb, gc_sb, gc_sb)
                for h in range(2):
                    c = 2 * pr + h
                    nc.tensor.matmul(
                        yt_ps,
                        lhsT=rs_sb[:, c * N : (c + 1) * N],
                        rhs=g2_sb[:, h * 512 : (h + 1) * 512],
                        start=(c == 0),
                        stop=False,
                    )
            # linear + const part (ones row of xt65)
            nc.tensor.matmul(
                yt_ps,
                lhsT=l65_sb,
                rhs=xt65[:, c0 : c0 + 512],
                start=False,
                stop=True,
            )
            yt_sb = ysb.tile([N, 512], F32)
            nc.vector.tensor_copy(out=yt_sb, in_=yt_ps)

            for tt in range(4):
                yb_ps = ybps.tile([P, N], F32)
                nc.tensor.transpose(
                    yb_ps, yt_sb[:, tt * P : (tt + 1) * P], ident[0:N, 0:N]
                )
                st_sb = stat.tile([P, 6], F32)
                nc.vector.bn_stats(out=st_sb, in_=yb_ps)
                mv_sb = stat.tile([P, 2], F32)
                nc.vector.bn_aggr(out=mv_sb, in_=st_sb)
                sd_sb = stat.tile([P, 1], F32)
                nc.scalar.activation(
                    out=sd_sb,
                    in_=mv_sb[:, 1:2],
                    func=mybir.ActivationFunctionType.Sqrt,
                    bias=eps_sb,
                    scale=1.0,
                )
                rq_sb = stat.tile([P, 1], F32)
                nc.vector.reciprocal(out=rq_sb, in_=sd_sb)
                o_sb = opool.tile([P, N], F32)
                nc.vector.tensor_scalar(
                    out=o_sb,
                    in0=yb_ps,
                    scalar1=mv_sb[:, 0:1],
                    scalar2=rq_sb,
                    op0=mybir.AluOpType.subtract,
                    op1=mybir.AluOpType.mult,
                )
                if apply_gamma:
                    nc.vector.tensor_mul(o_sb, o_sb, g_sb)
                if apply_beta:
                    nc.vector.tensor_add(o_sb, o_sb, b_sb)
                r0 = c0 + tt * P
                nc.sync.dma_start(out=out[r0 : r0 + P, :], in_=o_sb)

    if not nc.is_finalized():
        nc.finalize()
    return nc


NPAIR = NX * (NX - 1) // 2  # 2016 pairwise sums
NQ = NPAIR + NX             # 2080 squared components (exact rank floor)
NCHUNK = 17                 # ceil(2080 / 128)
NQP = NCHUNK * P            # 2176 padded

# chunk units: 8 pairs + 1 single; per-unit eviction engine assignment
V4_UNITS = [(0, 1), (2, 3), (4, 5), (6, 7), (8, 9), (10, 11), (12, 13),
            (14, 15), (16,)]
V4_DVE_UNITS = frozenset({2, 6, 8})  # evict via DVE copy+mul; rest ScalarE
V4_SKEW = 3                          # units between G matmul and its acc


def _build_bass_v4(rows_per_core, apply_gamma, apply_beta,
                   dve_units=V4_DVE_UNITS, skew=V4_SKEW):
    """Polarization-pairs formulation, batch-major accumulation.

    Host rewrites y_quad_j = sum_{a<c} S_jac*(x_a+x_c)^2 + sum_a D_ja*x_a^2
    (S = sym quadratic form, D the corrected diagonal), so only 2080 squared
    linear forms are needed (vs 4096 for per-j eigendecomposition).  On
    device, per 512-row block:
      G [2176, 512] = T^T @ XT          17 matmuls      (TensorE, fp16)
      G2 = G^2  evicted PSUM->SBUF      ScalarE / DVE split
      y[batch, j] += G2_slice^T @ R     17x4 matmuls w/ G2 *stationary*
                                        (64-col moving R  -> batch-major y,
                                        no output transposes, ldweights free)
      y += XT65_slice^T @ L65           linear + const via host ones row
      LayerNorm on [128, 64] subtiles   (DVE/ScalarE)
    x arrives host-transposed (xt [65, rows] fp16 with ones row) so no
    on-device transposes exist at all.
    """
    ntiles = rows_per_core // P
    nblocks = rows_per_core // 512
    nc = bacc.Bacc(None, target_bir_lowering=False)

    xt = nc.declare_dram_parameter("xt", [NX + 1, rows_per_core], FP16,
                                   isOutput=False)
    tq = nc.declare_dram_parameter("tq", [NX, NQP], FP16, isOutput=False)
    rq = nc.declare_dram_parameter("rq", [P, NCHUNK * N], FP16, isOutput=False)
    l65 = nc.declare_dram_parameter("l65", [NX + 1, N], FP16, isOutput=False)
    if apply_gamma:
        gm = nc.declare_dram_parameter("gm", [P, N], F32, isOutput=False)
    if apply_beta:
        bt = nc.declare_dram_parameter("bt", [P, N], F32, isOutput=False)
    out = nc.declare_dram_parameter("out", [rows_per_core, N], F32, isOutput=True)

    with (
        tile.TileContext(nc) as tc,
        tc.tile_pool(name="consts", bufs=1) as consts,
        tc.tile_pool(name="xtp", bufs=3) as xtp,
        tc.tile_pool(name="g2pool", bufs=6) as g2pool,
        tc.tile_pool(name="opool", bufs=4) as opool,
        tc.tile_pool(name="stat", bufs=8) as stat,
        tc.tile_pool(name="gps", bufs=3, space="PSUM") as gps,
        tc.tile_pool(name="ybps", bufs=2, space="PSUM") as ybps,
    ):
        # const DMAs split into pieces so block 0 can start after ~1KB
        tq_sb = consts.tile([NX, NQP], FP16)
        tq_cuts = [0, 2 * P, 8 * P, NQP]
        for lo, hi in zip(tq_cuts[:-1], tq_cuts[1:]):
            nc.sync.dma_start(out=tq_sb[:, lo:hi], in_=tq[:, lo:hi])
        rq_sb = consts.tile([P, NCHUNK * N], FP16)
        rq_cuts = [0, 2 * N, 8 * N, NCHUNK * N]
        for lo, hi in zip(rq_cuts[:-1], rq_cuts[1:]):
            nc.sync.dma_start(out=rq_sb[:, lo:hi], in_=rq[:, lo:hi])
        l65_sb = consts.tile([NX + 1, N], FP16)
        nc.sync.dma_start(out=l65_sb, in_=l65[:, :])
        if apply_gamma:
            g_sb = consts.tile([P, N], F32)
            nc.sync.dma_start(out=g_sb, in_=gm[:, :])
        if apply_beta:
            b_sb = consts.tile([P, N], F32)
            nc.sync.dma_start(out=b_sb, in_=bt[:, :])
        eps_sb = consts.tile([P, 1], F32)
        nc.vector.memset(eps_sb, EPS)

        # warm-up absorbers: PE observes each const DMA once via single-dep
        # matmuls so steady-state instructions carry <=1 fresh cross-engine
        # wait (see v3 notes on walrus wait legalization).
        for lo in tq_cuts[:-1]:
            scr = gps.tile([1, 1], F32, tag="g")
            nc.tensor.matmul(scr, lhsT=tq_sb[:, lo : lo + 1],
                             rhs=tq_sb[:, lo : lo + 1], start=True, stop=True)
        for lo in rq_cuts[:-1]:
            scr = gps.tile([1, 1], F32, tag="g")
            nc.tensor.matmul(scr, lhsT=rq_sb[:, lo : lo + 1],
                             rhs=rq_sb[:, lo : lo + 1], start=True, stop=True)
        scr_m3 = gps.tile([1, 1], F32, tag="g")
        nc.tensor.matmul(scr_m3, lhsT=l65_sb[:, 0:1], rhs=l65_sb[:, 0:1],
                         start=True, stop=True)
        if apply_gamma:
            scr_v = consts.tile([P, 1], F32)
            nc.vector.tensor_copy(scr_v, g_sb[:, 0:1])
        if apply_beta:
            scr_v2 = consts.tile([P, 1], F32)
            nc.vector.tensor_copy(scr_v2, b_sb[:, 0:1])

        def emit_ln_stats(yb, c0):
            """DVE stats for all 4 subtiles, then one batched Act sqrt run
            (single act-table swap); returns per-subtile (mv, sd)."""
            mvsd = []
            for i in range(4):
                st_sb = stat.tile([P, 6], F32)
                nc.vector.bn_stats(out=st_sb, in_=yb[:, i * N : (i + 1) * N])
                mv_sb = stat.tile([P, 2], F32)
                nc.vector.bn_aggr(out=mv_sb, in_=st_sb)
                mvsd.append(mv_sb)
            sds = []
            for i in range(4):
                sd_sb = stat.tile([P, 1], F32)
                nc.scalar.activation(
                    out=sd_sb,
                    in_=mvsd[i][:, 1:2],
                    func=mybir.ActivationFunctionType.Sqrt,
                    bias=eps_sb,
                    scale=1.0,
                )
                sds.append(sd_sb)
            out_list = []
            for i in range(4):
                rs_sb = stat.tile([P, 1], F32)
                nc.vector.reciprocal(out=rs_sb, in_=sds[i])
                out_list.append((mvsd[i], rs_sb))
            return out_list

        def emit_ln_apply(yb, c0, mvsd):
            for i in range(4):
                mv_sb, rs_sb = mvsd[i]
                o_sb = opool.tile([P, N], F32)
                nc.vector.tensor_scalar(
                    out=o_sb,
                    in0=yb[:, i * N : (i + 1) * N],
                    scalar1=mv_sb[:, 0:1],
                    scalar2=rs_sb,
                    op0=mybir.AluOpType.subtract,
                    op1=mybir.AluOpType.mult,
                )
                if apply_gamma:
                    nc.vector.tensor_mul(o_sb, o_sb, g_sb)
                if apply_beta:
                    nc.vector.tensor_add(o_sb, o_sb, b_sb)
                r0 = c0 + i * P
                nc.sync.dma_start(out=out[r0 : r0 + P, :], in_=o_sb)

        ln_prev = None  # (yb, c0) of previous block, LN'd during this one

        for s in range(nblocks):
            c0 = s * 512
            xt_sb = xtp.tile([NX + 1, 512], FP16)
            nc.sync.dma_start(out=xt_sb, in_=xt[:, c0 : c0 + 512])

            # full-bank tile: yb owns its 2KB PSUM zero region.  Exactly one
            # accumulation group spans all 4 subtiles: the FIRST matmul's
            # start=True marks the whole bank pending-zero (so each subtile's
            # first write overwrites), the LAST linear matmul stops the group.
            yb = ybps.tile([P, 512], F32)
            first_acc = [True]

            def emit_acc(unit, g2):
                for h, c in enumerate(unit):
                    for i in range(4):
                        nc.tensor.matmul(
                            yb[:, i * N : (i + 1) * N],
                            lhsT=g2[:, h * 512 + i * P : h * 512 + (i + 1) * P],
                            rhs=rq_sb[:, c * N : (c + 1) * N],
                            start=first_acc[0],
                            stop=False,
                        )
                        first_acc[0] = False

            pending = []
            mvsd = None
            for u, unit in enumerate(V4_UNITS):
                w = 512 * len(unit)
                g_ps = gps.tile([P, w], F32, tag="g")
                for h, c in enumerate(unit):
                    nc.tensor.matmul(
                        g_ps[:, h * 512 : (h + 1) * 512],
                        lhsT=tq_sb[:, c * P : (c + 1) * P],
                        rhs=xt_sb[0:NX, :],
                        start=True,
                        stop=True,
                    )
                g2 = g2pool.tile([P, w], G2DT)
                if u in dve_units:
                    gc = g2pool.tile([P, w], G2DT, tag="gc")
                    nc.vector.tensor_copy(out=gc, in_=g_ps)
                    nc.vector.tensor_mul(g2, gc, gc)
                else:
                    nc.scalar.square(g2, g_ps)
                pending.append((unit, g2))
                if u >= skew:
                    emit_acc(*pending[u - skew])
                if u == 4 and ln_prev is not None:
                    mvsd = emit_ln_stats(*ln_prev)
                if u == 6 and ln_prev is not None:
                    emit_ln_apply(*ln_prev, mvsd)
            for u in range(len(V4_UNITS) - skew, len(V4_UNITS)):
                emit_acc(*pending[u])

            # linear + const (host ones row of xt); last one closes the group
            for i in range(4):
                nc.tensor.matmul(
                    yb[:, i * N : (i + 1) * N],
                    lhsT=xt_sb[:, i * P : (i + 1) * P],
                    rhs=l65_sb,
                    start=False,
                    stop=(i == 3),
                )
            ln_prev = (yb, c0)

        mvsd = emit_ln_stats(*ln_prev)
        emit_ln_apply(*ln_prev, mvsd)

    if not nc.is_finalized():
        nc.finalize()
    return nc


def _host_constants_v4(W, b):
    A = np.asarray(W, np.float64).reshape(N, NX + 1, NX + 1)  # [j, a, c]
    Q = A[:, 1:, 1:]
    S = 0.5 * (Q + Q.transpose(0, 2, 1))  # [j, a, c]
    pa, pc = np.triu_indices(NX, k=1)     # 2016 pairs a<c
    tqm = np.zeros((NX, NQP), np.float64)
    tqm[pa, np.arange(NPAIR)] = 1.0
    tqm[pc, np.arange(NPAIR)] = 1.0
    tqm[np.arange(NX), NPAIR + np.arange(NX)] = 1.0
    rqm = np.zeros((NQP, N), np.float64)
    rqm[np.arange(NPAIR), :] = S[:, pa, pc].T
    rowsum = S.sum(axis=2)                 # [j, a]
    diag = np.einsum("jaa->ja", S)
    rqm[NPAIR + np.arange(NX), :] = (2.0 * diag - rowsum).T
    rq128 = np.ascontiguousarray(
        rqm.reshape(NCHUNK, P, N).transpose(1, 0, 2).reshape(P, NCHUNK * N)
    ).astype(np.float16)
    Lw = (A[:, 0, 1:] + A[:, 1:, 0]).T     # [c, j]
    cvec = A[:, 0, 0] + np.asarray(b, np.float64)
    l65c = np.ascontiguousarray(
        np.concatenate([Lw, cvec[None, :]], axis=0)
    ).astype(np.float16)
    return tqm.astype(np.float16), rq128, l65c


def _g2_is_bf16():
    return G2DT == BF16


def _host_constants_v3(W, b):
    import ml_dtypes

    A = np.asarray(W, np.float64).reshape(N, NX + 1, NX + 1)  # [j, a, c]
    Q = A[:, 1:, 1:]
    S = 0.5 * (Q + Q.transpose(0, 2, 1))
    lam, U = np.linalg.eigh(S)  # lam [j, k], U [j, c, k]
    U2 = U * np.sqrt(np.abs(lam))[:, None, :]  # [j, c, k]
    # u2[c, j*64 + k]
    u2 = np.ascontiguousarray(
        U2.transpose(1, 0, 2).reshape(NX, N * NX)
    ).astype(np.float16)
    # rsel chunks: rsel[:, t*64:(t+1)*64][r, j'] = sign(lam[j, k]) * (j == j')
    # where jk = t*128 + r, j = jk // 64, k = jk % 64
    sgn = np.sign(lam)  # [j, k]
    rsel = np.zeros((32, P, N), np.float32)
    jk = np.arange(N * NX)
    jj = jk // NX
    kk = jk % NX
    rsel[jk // P, jk % P, jj] = sgn[jj, kk]
    rsel = np.ascontiguousarray(rsel.transpose(1, 0, 2).reshape(P, 32 * N)).astype(
        np.float16
    )
    # l65: rows 0..63 linear weights, row 64 const (incl. bias)
    Lw = (A[:, 0, 1:] + A[:, 1:, 0]).T  # [c, j]
    cvec = A[:, 0, 0] + np.asarray(b, np.float64)
    l65c = np.concatenate([Lw, cvec[None, :]], axis=0).astype(np.float16)
    return u2, rsel, np.ascontiguousarray(l65c)


def _get_bass(rows_per_core, apply_gamma, apply_beta, version=4):
    key = (rows_per_core, apply_gamma, apply_beta, version)
    if key not in _CACHE:
        if version == 4:
            _CACHE[key] = _build_bass_v4(rows_per_core, apply_gamma, apply_beta)
        elif version == 3:
            _CACHE[key] = _build_bass_v3(rows_per_core, apply_gamma, apply_beta)
        else:
            _CACHE[key] = _build_bass(rows_per_core, apply_gamma, apply_beta)
    return _CACHE[key]


def _host_constants(W, b, gamma, beta):
    A = np.asarray(W, np.float32).reshape(N, NX + 1, NX + 1)  # [j, a, c]
    # V[c, j*64 + a] = A[j, a+1, c+1]
    import ml_dtypes
    V = np.ascontiguousarray(
        A[:, 1:, 1:].transpose(2, 0, 1).reshape(NX, N * NX)
    ).astype(ml_dtypes.bfloat16)
    # L[c, j] = A[j, 0, c+1] + A[j, c+1, 0]
    L = np.ascontiguousarray((A[:, 0, 1:] + A[:, 1:, 0]).T).astype(ml_dtypes.bfloat16)
    # const per j (+ linear bias), broadcast to all 128 partitions
    cvec = A[:, 0, 0] + np.asarray(b, np.float32)
    C = np.ascontiguousarray(np.tile(cvec[None, :], (P, 1))).astype(np.float32)
    G = np.ascontiguousarray(np.tile(np.asarray(gamma, np.float32)[None, :], (P, 1)))
    Bt = np.ascontiguousarray(np.tile(np.asarray(beta, np.float32)[None, :], (P, 1)))
    return V, L, C, G, Bt


def _make_in_maps(x, W, b, gamma, beta, rows_per_core, version=4):
    x = np.ascontiguousarray(np.asarray(x, np.float32))
    apply_gamma = not np.all(np.asarray(gamma) == 1.0)
    apply_beta = not np.all(np.asarray(beta) == 0.0)
    G = np.ascontiguousarray(np.tile(np.asarray(gamma, np.float32)[None, :], (P, 1)))
    Bt = np.ascontiguousarray(np.tile(np.asarray(beta, np.float32)[None, :], (P, 1)))
    in_maps = []
    if version == 4:
        tqm, rq128, l65c = _host_constants_v4(W, b)
        n_used = rows_per_core * N_CORES
        xt_all = np.empty((NX + 1, n_used), np.float16)
        xt_all[0:NX, :] = x[:n_used].T
        xt_all[NX, :] = 1.0
        for c in range(N_CORES):
            m = {
                "xt": np.ascontiguousarray(
                    xt_all[:, c * rows_per_core : (c + 1) * rows_per_core]
                ),
                "tq": tqm,
                "rq": rq128,
                "l65": l65c,
            }
            if apply_gamma:
                m["gm"] = G
            if apply_beta:
                m["bt"] = Bt
            in_maps.append(m)
    elif version == 3:
        u2, rsel, l65c = _host_constants_v3(W, b)
        for c in range(N_CORES):
            m = {
                "xs": np.ascontiguousarray(
                    x[c * rows_per_core : (c + 1) * rows_per_core, :]
                ),
                "u2": u2,
                "rsel": rsel,
                "l65": l65c,
            }
            if apply_gamma:
                m["gm"] = G
            if apply_beta:
                m["bt"] = Bt
            in_maps.append(m)
    else:
        V, L, C, _, _ = _host_constants(W, b, gamma, beta)
        for c in range(N_CORES):
            m = {
                "xs": np.ascontiguousarray(
                    x[c * rows_per_core : (c + 1) * rows_per_core, :]
                ),
                "vq": V,
                "lw": L,
                "cb": C,
            }
            if apply_gamma:
                m["gm"] = G
            if apply_beta:
                m["bt"] = Bt
            in_maps.append(m)
    return in_maps, apply_gamma, apply_beta


def kernel(x, W, b, gamma, beta, _rows_per_core=ROWS_PER_CORE, _trace=False,
           _version=4):
    in_maps, apply_gamma, apply_beta = _make_in_maps(
        x, W, b, gamma, beta, _rows_per_core, _version
    )
    nc = _get_bass(_rows_per_core, apply_gamma, apply_beta, _version)
    res = run_bass_kernel_spmd(
        nc, in_maps, core_ids=list(range(N_CORES)), trace=_trace
    )
    outs = [res.results[i]["out"] for i in range(N_CORES)]
    full = np.concatenate(outs, axis=0)  # [8 * rows_per_core, 64]
    if _trace:
        kernel._last_result = res
    return full.reshape(-1, 8, 8).astype(np.float32)

